# revision 13
# baseline (speedup 1.0000x reference)
"""Self-contained Trainium2 Bass kernel for nn_BigramLanguageModel.

8 NeuronCores = 4 samples x 2-way tensor parallel (heads 6+6, FFN 4608+4608,
vocab 16000+16000). Residual stream kept transposed (xT [C, T]) so every
matmul is transpose-free; LN stats via ones-matmul; bf16 matmul inputs with
f32 PSUM accumulation; pair AllReduce for proj/FFN partials; online CE
sum-exp on device, final CE combine + output assembly on host.
"""
import numpy as np
import ml_dtypes
import bass_rust
import jax
import concourse.bass as bass
import concourse.mybir as mybir
import concourse.tile as tile
from concourse.masks import make_identity
from concourse.vector_clock import ScopedClock

F32 = mybir.dt.float32
BF16 = mybir.dt.bfloat16
I32 = mybir.dt.int32
BF = ml_dtypes.bfloat16

# model constants (hardcoded per problem spec)
V, C, T, H, HS = 32000, 768, 1024, 12, 64
FF, NBLK, B = 9216, 3, 4
EPS = 1e-5
SCALE = C ** -0.5
NC_ = 8            # cores
HPC = H // 2       # heads per core = 6
DPC = HPC * HS     # head dims per core = 384
FPC = FF // 2      # ffn hidden per core = 4608
VPC = V // 2       # vocab per core = 16000
CC = C // 128      # 6 c-chunks
FC = FPC // 128    # 36 f-chunks
TC2 = T // 512     # 2 t-halves
TC8 = T // 128     # 8 t-chunks
VCH = 500          # vocab chunk for lm_head
NVC = VPC // VCH   # 32 vocab chunks
GROUPS = [[0, 1], [2, 3], [4, 5], [6, 7]]


class _TC(tile.TileContext):
    """TileContext whose tail avoids Drain{waits} (walrus: <=1 wait/inst)."""

    def _drain_and_barrier(self, tick_clock, wait_clock):
        gc = list(tick_clock.global_clock)
        for i, v in enumerate(gc):
            if not v:
                continue
            part = [v if j == i else 0 for j in range(len(gc))]
            n = self.nc.sync.nop(nofuse=True, hint="tail_wait")
            wait_clock.add_sem_waits(
                n.ins, ScopedClock({None: bass_rust.VectorClock(part)})
            )
        self.nc.sync.drain()
        self.nc.all_engine_barrier(sem_only=True)
        popped = self.nc._tile_sem_poison_stack.pop()
        assert popped is self._sem_poison
        self.nc.clear_and_free_semaphores(list(self.sems.allocated().values()))
        self.nc.all_engine_barrier(sem_only=True)


def _split_excess_waits(nc):
    """Hoist all but one sync wait per instruction onto same-engine NOPs."""
    n_new = 0
    for f in nc.m.functions:
        for bb in f.blocks:
            insts = list(bb.instructions)
            out = []
            changed = False
            for inst in insts:
                si = getattr(inst, "sync_info", None)
                waits = list(si.on_wait) if si is not None and si.on_wait else []
                keep = 0 if isinstance(inst, mybir.InstDrain) else 1
                if len(waits) > keep:
                    excess = waits[:-keep] if keep else waits
                    kept = waits[-keep:] if keep else []
                    for w in excess:
                        n_new += 1
                        out.append(mybir.InstNoOp(
                            name=f"wsplit-{n_new}-{inst.name}",
                            engine=inst.engine,
                            bass_nofuse=True,
                            sync_info=mybir.SyncInfo(on_wait=[w], on_update=[]),
                        ))
                    si.on_wait = kept
                    changed = True
                out.append(inst)
            if changed:
                bb.instructions.clear()
                bb.instructions.extend(out)
    return n_new


def build_program():
    nc = bass.Bass()

    tok = nc.dram_tensor("tok", [V, C], F32, kind="ExternalInput")
    x_idx = nc.dram_tensor("x_idx", [T, 1], I32, kind="ExternalInput")
    pos_t = nc.dram_tensor("pos_t", [128, CC, T], F32, kind="ExternalInput")
    wq_s = nc.dram_tensor("wq_s", [NBLK, 128, CC, DPC], BF16, kind="ExternalInput")
    wk_s = nc.dram_tensor("wk_s", [NBLK, 128, CC, DPC], BF16, kind="ExternalInput")
    wv_s = nc.dram_tensor("wv_s", [NBLK, 128, CC, DPC], BF16, kind="ExternalInput")
    wp_s = nc.dram_tensor("wp_s", [NBLK, 128, 3, C], BF16, kind="ExternalInput")
    g1_t = nc.dram_tensor("g1_t", [NBLK, 128, CC], F32, kind="ExternalInput")
    b1_t = nc.dram_tensor("b1_t", [NBLK, 128, CC], F32, kind="ExternalInput")
    g2_t = nc.dram_tensor("g2_t", [NBLK, 128, CC], F32, kind="ExternalInput")
    b2_t = nc.dram_tensor("b2_t", [NBLK, 128, CC], F32, kind="ExternalInput")
    bp_t = nc.dram_tensor("bp_t", [NBLK, 128, CC], F32, kind="ExternalInput")
    wf1_s = nc.dram_tensor("wf1_s", [NBLK, FC, 128, CC, 128], BF16, kind="ExternalInput")
    bf1_s = nc.dram_tensor("bf1_s", [NBLK, FC, 128, 1], F32, kind="ExternalInput")
    wf2_s = nc.dram_tensor("wf2_s", [NBLK, FC, 128, C], BF16, kind="ExternalInput")
    bf2_t = nc.dram_tensor("bf2_t", [NBLK, 128, CC], F32, kind="ExternalInput")
    gf_t = nc.dram_tensor("gf_t", [128, CC], F32, kind="ExternalInput")
    bff_t = nc.dram_tensor("bff_t", [128, CC], F32, kind="ExternalInput")
    wlm_s = nc.dram_tensor("wlm_s", [128, CC, VPC], BF16, kind="ExternalInput")
    blm_s = nc.dram_tensor("blm_s", [1, VPC], BF16, kind="ExternalInput")

    logits_o = nc.dram_tensor("logits_o", [T, VPC], F32, kind="ExternalOutput")
    rowsum_o = nc.dram_tensor("rowsum_o", [T, 1], F32, kind="ExternalOutput")

    with _TC(nc) as tc:
        _emit(nc, tc, locals())
    _split_excess_waits(nc)
    return nc


def _emit(nc, tc, d):
    from contextlib import ExitStack
    ctx = ExitStack()
    with ctx:
        sb = ctx.enter_context(tc.tile_pool(name="sb", bufs=1))
        sbr = ctx.enter_context(tc.tile_pool(name="sbr", bufs=2))
        wpool = ctx.enter_context(tc.tile_pool(name="wpool", bufs=2))
        psA = ctx.enter_context(tc.tile_pool(name="psA", bufs=2, space="PSUM"))
        dram = ctx.enter_context(tc.tile_pool(name="dram", bufs=2, space="DRAM"))

        # ---- constants ----
        identf = sb.tile([128, 128], F32)
        make_identity(nc, identf[:])
        ones_c = sb.tile([128, 1], BF16)
        nc.gpsimd.memset(ones_c[:], 1.0)
        ones1 = sb.tile([1, 128], BF16)
        nc.gpsimd.memset(ones1[:], 1.0)
        eps_t = sb.tile([1, 1], F32)
        nc.gpsimd.memset(eps_t[:], EPS)
        masks = sb.tile([128, 4, 512], BF16)
        for j in range(4):
            nc.gpsimd.memset(masks[:, j, :], 1.0)
            nc.gpsimd.affine_select(
                out=masks[:, j, :], in_=masks[:, j, :],
                compare_op=mybir.AluOpType.is_ge, fill=0.0,
                base=-128 * j, pattern=[[1, 512]], channel_multiplier=-1,
            )

        # ---- persistent activations ----
        xT = sb.tile([128, CC, T], F32)          # residual stream, transposed
        hT = sb.tile([128, CC, T], BF16)         # LN output (reused per LN)
        qT = sb.tile([128, 3, T], BF16)
        kT = sb.tile([128, 3, T], BF16)
        oT = sb.tile([128, 3, T], BF16)
        v_all = sb.tile([128, TC8, HPC, HS + 1], BF16)

        # ---- embedding: gather + transpose + pos add ----
        nc.sync.dma_start(xT[:], d["pos_t"][:])
        for tc8 in range(TC8):
            t0 = tc8 * 128
            idx_t = sbr.tile([128, 1], I32)
            nc.sync.dma_start(idx_t[:], d["x_idx"][t0:t0 + 128, :])
            xg = sbr.tile([128, C], F32)
            nc.gpsimd.indirect_dma_start(
                out=xg[:], out_offset=None, in_=d["tok"][:],
                in_offset=bass.IndirectOffsetOnAxis(ap=idx_t[:, :1], axis=0),
            )
            for cc in range(CC):
                ptr = psA.tile([128, 128], F32, tag="ps_small")
                nc.tensor.transpose(ptr[:], xg[:, cc * 128:cc * 128 + 128], identf[:])
                nc.vector.tensor_add(
                    xT[:, cc, t0:t0 + 128], xT[:, cc, t0:t0 + 128], ptr[:])

        def layer_norm(g_ap, b_ap, out_bf):
            """LN over c (partition-dim) of xT -> out_bf (bf16), transposed form."""
            gb_t = sbr.tile([128, CC], F32, tag="gb_g")
            nc.sync.dma_start(gb_t[:], g_ap)
            bb_t = sbr.tile([128, CC], F32, tag="gb_b")
            nc.sync.dma_start(bb_t[:], b_ap)
            srow = sbr.tile([1, T], F32, tag="srow", bufs=1)
            qrow = sbr.tile([1, T], F32, tag="qrow", bufs=1)
            for th in range(TC2):
                u0 = th * 512
                ps_s = psA.tile([1, 512], F32, tag="ps_small")
                ps_q = psA.tile([1, 512], F32, tag="ps_small")
                for cc in range(CC):
                    xbf = sbr.tile([128, 512], BF16, tag="xbf")
                    nc.vector.tensor_copy(xbf[:], xT[:, cc, u0:u0 + 512])
                    sqbf = sbr.tile([128, 512], BF16, tag="sqbf")
                    nc.vector.tensor_mul(sqbf[:], xbf[:], xbf[:])
                    nc.tensor.matmul(ps_s[:], ones_c[:], xbf[:],
                                     start=(cc == 0), stop=(cc == CC - 1))
                    nc.tensor.matmul(ps_q[:], ones_c[:], sqbf[:],
                                     start=(cc == 0), stop=(cc == CC - 1))
                nc.scalar.mul(srow[:, u0:u0 + 512], ps_s[:], 1.0 / C)
                nc.scalar.mul(qrow[:, u0:u0 + 512], ps_q[:], 1.0 / C)
            m2 = sbr.tile([1, T], F32, tag="m2", bufs=1)
            nc.vector.tensor_mul(m2[:], srow[:], srow[:])
            nc.vector.tensor_tensor(
                out=qrow[:], in0=qrow[:], in1=m2[:],
                op=mybir.AluOpType.subtract)           # qrow = var
            nc.scalar.activation(m2[:], qrow[:],
                                 mybir.ActivationFunctionType.Sqrt,
                                 bias=eps_t[:])        # m2 = std
            nc.vector.reciprocal(qrow[:], m2[:])       # qrow = rstd
            mean_bf = sbr.tile([1, T], BF16, tag="mean_bf", bufs=1)
            nc.vector.tensor_copy(mean_bf[:], srow[:])
            rstd_bf = sbr.tile([1, T], BF16, tag="rstd_bf", bufs=1)
            nc.vector.tensor_copy(rstd_bf[:], qrow[:])
            for th in range(TC2):
                u0 = th * 512
                ps_m = psA.tile([128, 512], F32, tag="ps_small")
                nc.tensor.matmul(ps_m[:], ones1[:], mean_bf[:, u0:u0 + 512],
                                 start=True, stop=True)
                ps_r = psA.tile([128, 512], F32, tag="ps_small")
                nc.tensor.matmul(ps_r[:], ones1[:], rstd_bf[:, u0:u0 + 512],
                                 start=True, stop=True)
                for cc in range(CC):
                    t1 = sbr.tile([128, 512], F32, tag="lnt1")
                    nc.vector.tensor_tensor(
                        out=t1[:], in0=xT[:, cc, u0:u0 + 512], in1=ps_m[:],
                        op=mybir.AluOpType.subtract)
                    t2 = sbr.tile([128, 512], F32, tag="lnt2")
                    nc.vector.tensor_mul(t2[:], t1[:], ps_r[:])
                    nc.scalar.activation(
                        out_bf[:, cc, u0:u0 + 512], t2[:],
                        mybir.ActivationFunctionType.Identity,
                        bias=bb_t[:, cc:cc + 1], scale=gb_t[:, cc:cc + 1])

        def allreduce_add(partial_bf, bias_ap):
            """Pair-AllReduce partial [128,CC,T] bf16; add result+bias into xT."""
            bias_sb = sbr.tile([128, CC], F32, tag="ar_bias")
            nc.sync.dma_start(bias_sb[:], bias_ap)
            in_b = dram.tile([128, CC, T], BF16, tag="arb")
            out_b = dram.tile([128, CC, T], BF16, tag="arb")
            nc.sync.dma_start(in_b[:], partial_bf[:])
            nc.gpsimd.collective_compute(
                "AllReduce", mybir.AluOpType.add, replica_groups=GROUPS,
                ins=[in_b.opt()], outs=[out_b.opt()],
            )
            arr = sbr.tile([128, CC, T], BF16, tag="arpart")
            nc.sync.dma_start(arr[:], out_b[:])
            for cc in range(CC):
                for th in range(TC2):
                    u0 = th * 512
                    tb = sbr.tile([128, 512], F32, tag="artb")
                    nc.scalar.activation(
                        tb[:], arr[:, cc, u0:u0 + 512],
                        mybir.ActivationFunctionType.Identity,
                        bias=bias_sb[:, cc:cc + 1])
                    nc.vector.tensor_add(
                        xT[:, cc, u0:u0 + 512], xT[:, cc, u0:u0 + 512], tb[:])

        # ================= transformer blocks =================
        for i in range(NBLK):
            layer_norm(d["g1_t"][i], d["b1_t"][i], hT)

            # weights for this block
            wq_t = wpool.tile([128, CC, DPC], BF16, tag="wq", bufs=1)
            nc.sync.dma_start(wq_t[:], d["wq_s"][i])
            wk_t = wpool.tile([128, CC, DPC], BF16, tag="wk", bufs=1)
            nc.sync.dma_start(wk_t[:], d["wk_s"][i])
            wv_t = wpool.tile([128, CC, DPC], BF16, tag="wv", bufs=1)
            nc.sync.dma_start(wv_t[:], d["wv_s"][i])
            wp_t = wpool.tile([128, 3, C], BF16, tag="wp", bufs=1)
            nc.sync.dma_start(wp_t[:], d["wp_s"][i])

            # qT, kT: [384, T] as 3 partition-chunks
            for dst, w_t in ((qT, wq_t), (kT, wk_t)):
                for dc in range(3):
                    for th in range(TC2):
                        u0 = th * 512
                        ps = psA.tile([128, 512], F32, tag="ps_small")
                        for cc in range(CC):
                            nc.tensor.matmul(
                                ps[:], w_t[:, cc, dc * 128:dc * 128 + 128],
                                hT[:, cc, u0:u0 + 512],
                                start=(cc == 0), stop=(cc == CC - 1))
                        nc.scalar.copy(dst[:, dc, u0:u0 + 512], ps[:])
            # v: per s-chunk [128, 384] -> per-head tiles with ones col
            for sc in range(TC8):
                s0 = sc * 128
                ps = psA.tile([128, DPC], F32, tag="ps_small")
                for cc in range(CC):
                    nc.tensor.matmul(ps[:], hT[:, cc, s0:s0 + 128], wv_t[:, cc, :],
                                     start=(cc == 0), stop=(cc == CC - 1))
                for hh in range(HPC):
                    nc.scalar.copy(v_all[:, sc, hh, 0:HS],
                                   ps[:, hh * HS:hh * HS + HS])
                    nc.gpsimd.memset(v_all[:, sc, hh, HS:HS + 1], 1.0)

            # attention per (head, t-half), flash-style over s-chunks
            with tc.tile_pool(name=f"psAtt{i}", bufs=1, space="PSUM") as psAtt:
                for hh in range(HPC):
                    dc, off = hh // 2, (hh % 2) * 64
                    for th in range(TC2):
                        u0 = th * 512
                        n_s = 4 if th == 0 else 8
                        po = psAtt.tile([HS + 1, 512], F32, tag="po", bufs=2)
                        for sc in range(n_s):
                            s0 = sc * 128
                            psc = psAtt.tile([128, 512], F32, tag="psc", bufs=2)
                            nc.tensor.matmul(
                                psc[:], kT[off:off + 64, dc, s0:s0 + 128],
                                qT[off:off + 64, dc, u0:u0 + 512],
                                start=True, stop=True)
                            ebf = sbr.tile([128, 512], BF16, tag="ebf", bufs=3)
                            nc.scalar.activation(
                                ebf[:], psc[:], mybir.ActivationFunctionType.Exp,
                                scale=SCALE)
                            j = sc - (0 if th == 0 else 4)
                            if j >= 0:
                                nc.vector.tensor_mul(ebf[:], ebf[:],
                                                     masks[:, j, :])
                            nc.tensor.matmul(
                                po[:], v_all[:, sc, hh, :], ebf[:],
                                start=(sc == 0), stop=(sc == n_s - 1))
                        recip = sbr.tile([1, 512], BF16, tag="recip")
                        with nc.allow_low_precision(
                                reason="softmax denom fits bf16"):
                            nc.vector.reciprocal(recip[:], po[HS:HS + 1, :])
                        prr = psAtt.tile([64, 512], F32, tag="psc", bufs=2)
                        nc.tensor.matmul(prr[:], ones1[:, 0:64], recip[:],
                                         start=True, stop=True)
                        rep = sbr.tile([64, 512], F32, tag="rep")
                        nc.scalar.copy(rep[:], prr[:])
                        nc.vector.tensor_mul(
                            oT[off:off + 64, dc, u0:u0 + 512], po[0:HS, :],
                            rep[:])

            # proj + pair-AllReduce + residual
            pp = sbr.tile([128, CC, T], BF16, tag="arpart")
            for cc in range(CC):
                for th in range(TC2):
                    u0 = th * 512
                    ps = psA.tile([128, 512], F32, tag="ps_small")
                    for ac in range(3):
                        nc.tensor.matmul(
                            ps[:], wp_t[:, ac, cc * 128:cc * 128 + 128],
                            oT[:, ac, u0:u0 + 512],
                            start=(ac == 0), stop=(ac == 2))
                    nc.scalar.copy(pp[:, cc, u0:u0 + 512], ps[:])
            allreduce_add(pp, d["bp_t"][i])

            # LN2 + FFN
            layer_norm(d["g2_t"][i], d["b2_t"][i], hT)
            with tc.tile_pool(name=f"psB{i}", bufs=1, space="PSUM") as psB:
                fout = sbr.tile([128, CC, T], BF16, tag="arpart")
                for th in range(TC2):
                    u0 = th * 512
                    pouts = [psB.tile([128, 512], F32, tag=f"pB{cc}",
                                      name=f"pB{cc}") for cc in range(CC)]
                    for fc in range(FC):
                        w1c = wpool.tile([128, CC, 128], BF16, tag="w1c", bufs=3)
                        nc.sync.dma_start(w1c[:], d["wf1_s"][i, fc])
                        w2c = wpool.tile([128, C], BF16, tag="w2c", bufs=3)
                        nc.sync.dma_start(w2c[:], d["wf2_s"][i, fc])
                        b1c = wpool.tile([128, 1], F32, tag="b1c", bufs=3)
                        nc.sync.dma_start(b1c[:], d["bf1_s"][i, fc])
                        pf = psA.tile([128, 512], F32, tag="ps_small")
                        for cc in range(CC):
                            nc.tensor.matmul(
                                pf[:], w1c[:, cc, :], hT[:, cc, u0:u0 + 512],
                                start=(cc == 0), stop=(cc == CC - 1))
                        fbf = sbr.tile([128, 512], BF16, tag="fbf", bufs=3)
                        nc.scalar.activation(
                            fbf[:], pf[:], mybir.ActivationFunctionType.Relu,
                            bias=b1c[:])
                        for cc in range(CC):
                            nc.tensor.matmul(
                                pouts[cc][:], w2c[:, cc * 128:cc * 128 + 128],
                                fbf[:], start=(fc == 0), stop=(fc == FC - 1))
                    for cc in range(CC):
                        nc.scalar.copy(fout[:, cc, u0:u0 + 512], pouts[cc][:])
                allreduce_add(fout, d["bf2_t"][i])

        # ================= final LN + lm_head + CE =================
        layer_norm(d["gf_t"][:], d["bff_t"][:], hT)
        srun = sb.tile([128, TC8], F32)
        for vc in range(NVC):
            v0 = vc * VCH
            wlm_t = wpool.tile([128, CC, VCH], BF16, tag="wlm", bufs=2)
            nc.sync.dma_start(wlm_t[:], d["wlm_s"][:, :, v0:v0 + VCH])
            blm_t = wpool.tile([1, VCH], BF16, tag="blmc", bufs=2)
            nc.sync.dma_start(blm_t[:], d["blm_s"][:, v0:v0 + VCH])
            for t8 in range(TC8):
                t0 = t8 * 128
                ps = psA.tile([128, VCH], F32, tag="ps_small")
                for cc in range(CC):
                    nc.tensor.matmul(ps[:], hT[:, cc, t0:t0 + 128], wlm_t[:, cc, :],
                                     start=(cc == 0), stop=False)
                nc.tensor.matmul(ps[:], ones1[:], blm_t[:],
                                 start=False, stop=True)
                lg = sbr.tile([128, VCH], F32, tag="lg", bufs=3)
                nc.scalar.copy(lg[:], ps[:])
                nc.sync.dma_start(d["logits_o"][t0:t0 + 128, v0:v0 + VCH], lg[:])
                et = sbr.tile([128, VCH], BF16, tag="et", bufs=3)
                sacc = sbr.tile([128, 1], F32, tag="sacc", bufs=3)
                nc.scalar.activation(et[:], lg[:],
                                     mybir.ActivationFunctionType.Exp,
                                     accum_out=sacc[:])
                if vc == 0:
                    nc.vector.tensor_copy(srun[:, t8:t8 + 1], sacc[:])
                else:
                    nc.vector.tensor_add(srun[:, t8:t8 + 1], srun[:, t8:t8 + 1],
                                         sacc[:])
        for t8 in range(TC8):
            nc.sync.dma_start(d["rowsum_o"][t8 * 128:(t8 + 1) * 128, :],
                              srun[:, t8:t8 + 1])


# ======================= host side =======================

_RUNNER = None


class _Runner:
    def __init__(self):
        from concourse import bass2jax
        from jax.sharding import Mesh, PartitionSpec
        from jax.experimental.shard_map import shard_map

        self.nc = build_program()
        bass2jax.install_neuronx_cc_hook()
        nc = self.nc
        pid_name = (nc.partition_id_tensor.name
                    if nc.partition_id_tensor else None)
        in_names, out_names, out_avals = [], [], []
        self.zero_shapes = []
        for alloc in nc.m.functions[0].allocations:
            if not isinstance(alloc, mybir.MemoryLocationSet):
                continue
            assert alloc.memorylocations
            name = alloc.memorylocations[0].name
            if alloc.kind == "ExternalInput":
                if name != pid_name:
                    in_names.append(name)
            elif alloc.kind == "ExternalOutput":
                shape = tuple(alloc.tensor_shape)
                dtype = mybir.dt.np(alloc.dtype)
                out_names.append(name)
                out_avals.append(jax.core.ShapedArray(shape, dtype))
                self.zero_shapes.append((shape, dtype))
        self.n_params = len(in_names)
        all_in = in_names + out_names
        if pid_name is not None:
            all_in.append(pid_name)
        self.in_names = all_in
        self.out_names = out_names
        self.param_names = in_names

        all_in_names = tuple(self.in_names)
        out_avals_t = tuple(out_avals)
        out_names_t = tuple(out_names)

        def _body(*args):
            operands = list(args)
            if pid_name is not None:
                operands.append(bass2jax.partition_id_tensor())
            outs = bass2jax._bass_exec_p.bind(
                *operands,
                out_avals=out_avals_t,
                in_names=all_in_names,
                out_names=out_names_t,
                lowering_input_output_aliases=(),
                sim_require_finite=True,
                sim_require_nnan=True,
                nc=nc,
            )
            return tuple(outs)

        devices = jax.devices()[:NC_]
        self.mesh = Mesh(np.asarray(devices), ("core",))
        n_out = len(out_names)
        donate = tuple(range(self.n_params, self.n_params + n_out))
        in_specs = (PartitionSpec("core"),) * (self.n_params + n_out)
        out_specs = (PartitionSpec("core"),) * n_out
        self.fn = jax.jit(
            shard_map(_body, mesh=self.mesh, in_specs=in_specs,
                      out_specs=out_specs, check_rep=False),
            donate_argnums=donate, keep_unused=True,
        )
        self._dev_inputs = None
        self._zeros_fn = None

    def stage_inputs(self, in_maps):
        from jax.sharding import NamedSharding, PartitionSpec
        sh = NamedSharding(self.mesh, PartitionSpec("core"))
        concat = [
            np.concatenate([np.asarray(in_maps[c][n]) for c in range(NC_)], axis=0)
            for n in self.param_names
        ]
        self._dev_inputs = [jax.device_put(a, sh) for a in concat]
        for a in self._dev_inputs:
            a.block_until_ready()

    def _make_zeros(self):
        import jax.numpy as jnp
        from jax.sharding import NamedSharding, PartitionSpec
        sh = NamedSharding(self.mesh, PartitionSpec("core"))
        if self._zeros_fn is None:
            shapes = [(tuple([NC_ * s[0]] + list(s[1:])), dt)
                      for s, dt in self.zero_shapes]
            self._zeros_fn = jax.jit(
                lambda: tuple(jnp.zeros(s, d) for s, d in shapes),
                out_shardings=tuple(sh for _ in shapes))
        return list(self._zeros_fn())

    def run(self):
        zeros = self._make_zeros()
        outs = self.fn(*self._dev_inputs, *zeros)
        return outs

    def results(self, outs):
        res = []
        for c in range(NC_):
            m = {}
            for i, name in enumerate(self.out_names):
                arr = np.asarray(outs[i])
                per = arr.shape[0] // NC_
                m[name] = arr[c * per:(c + 1) * per]
            res.append(m)
        return res


def _prep_in_maps(inputs):
    f = {k: np.asarray(v) for k, v in inputs.items()}
    tok = f["tok_emb"].astype(np.float32)
    pos_t = np.ascontiguousarray(
        f["pos_emb"][:T].astype(np.float32).T.reshape(CC, 128, T)
        .transpose(1, 0, 2))
    wq, wk, wv = f["wq"], f["wk"], f["wv"]
    wproj, wf1, wf2 = f["wproj"], f["wf1"], f["wf2"]
    wlm = f["wlm"]

    def qkv_slice(w, h):
        ws = w[:, h * HPC:(h + 1) * HPC]                 # [3, 6, 768, 64]
        ws = ws.transpose(0, 2, 1, 3).reshape(NBLK, C, DPC)
        return np.ascontiguousarray(
            ws.reshape(NBLK, CC, 128, DPC).transpose(0, 2, 1, 3)).astype(BF)

    def col_layout(a):  # [rows=768, X] -> [128, CC, X]
        return np.ascontiguousarray(
            a.reshape(CC, 128, -1).transpose(1, 0, 2))

    def bias_layout(bvec):  # [3, 768] -> [3, 128, CC]
        return np.ascontiguousarray(
            bvec.reshape(NBLK, CC, 128).transpose(0, 2, 1)).astype(np.float32)

    in_maps = []
    for c in range(NC_):
        b, h = c // 2, c % 2
        w1h = wf1[:, :, h * FPC:(h + 1) * FPC]           # [3, 768, 4608]
        wf1_l = np.ascontiguousarray(
            w1h.reshape(NBLK, CC, 128, FC, 128).transpose(0, 3, 2, 1, 4)
        ).astype(BF)
        w2h = wf2[:, h * FPC:(h + 1) * FPC, :]           # [3, 4608, 768]
        wf2_l = np.ascontiguousarray(
            w2h.reshape(NBLK, FC, 128, C)).astype(BF)
        wp_h = wproj[:, h * DPC:(h + 1) * DPC, :]        # [3, 384, 768]
        wp_l = np.ascontiguousarray(
            wp_h.reshape(NBLK, 3, 128, C).transpose(0, 2, 1, 3)).astype(BF)
        wlm_h = wlm[:, h * VPC:(h + 1) * VPC]            # [768, 16000]
        wlm_l = col_layout(wlm_h).astype(BF)
        in_maps.append({
            "tok": tok,
            "x_idx": f["sources"][b].astype(np.int32).reshape(T, 1),
            "pos_t": pos_t,
            "wq_s": qkv_slice(wq, h),
            "wk_s": qkv_slice(wk, h),
            "wv_s": qkv_slice(wv, h),
            "wp_s": wp_l,
            "g1_t": bias_layout(f["g1"]), "b1_t": bias_layout(f["b1"]),
            "g2_t": bias_layout(f["g2"]), "b2_t": bias_layout(f["b2"]),
            "bp_t": bias_layout(f["bproj"]),
            "wf1_s": wf1_l,
            "bf1_s": np.ascontiguousarray(
                f["bf1"][:, h * FPC:(h + 1) * FPC]
                .reshape(NBLK, FC, 128, 1)).astype(np.float32),
            "wf2_s": wf2_l,
            "bf2_t": bias_layout(f["bf2"]),
            "gf_t": np.ascontiguousarray(
                f["gf"].reshape(CC, 128).T).astype(np.float32),
            "bff_t": np.ascontiguousarray(
                f["bf"].reshape(CC, 128).T).astype(np.float32),
            "wlm_s": wlm_l,
            "blm_s": f["blm"][h * VPC:(h + 1) * VPC].reshape(1, VPC).astype(BF),
        })
    return in_maps


def _assemble(results, targets):
    logits2 = np.empty((B * T, V), np.float32)
    S = np.empty((B * T,), np.float64)
    for c in range(NC_):
        b, h = c // 2, c % 2
        logits2[b * T:(b + 1) * T, h * VPC:(h + 1) * VPC] = results[c]["logits_o"]
    for b in range(B):
        S[b * T:(b + 1) * T] = (
            results[2 * b]["rowsum_o"][:, 0].astype(np.float64)
            + results[2 * b + 1]["rowsum_o"][:, 0].astype(np.float64))
    lse = np.log(S)
    tgt = np.asarray(targets).reshape(-1).astype(np.int64)
    lt = logits2[np.arange(B * T), tgt].astype(np.float64)
    loss = np.float32(-(lt - lse).mean())
    return logits2, loss


def kernel(**inputs):
    global _RUNNER
    if _RUNNER is None:
        _RUNNER = _Runner()
    in_maps = _prep_in_maps(inputs)
    _RUNNER.stage_inputs(in_maps)
    outs = _RUNNER.run()
    results = _RUNNER.results(outs)
    return _assemble(results, inputs["targets"])
